# revision 106
# baseline (speedup 1.0000x reference)
"""Trainium2 Bass kernel for CustomEncoderWithAction (gnn_message_passing).

Strategy (8 NeuronCores, full inputs in / full output out):
  * Only pooled[robot_idx] (B=192 rows) is consumed downstream, so the
    [N,N] pairwise pooling is computed ONLY for the 192 robot agents,
    sharded 24 per core.
  * LSTM encoder (T=8, all N agents) replicated on every core, software-
    pipelined as 2 groups x 6 agent-tiles:
      - per group+step ONE gates matmul: stationary lhsT = transposed
        [h|x|1] block (one PE transpose per group), moving rhs = [109,384]
        block-column weight matrix (tile j's W_hh/W_xh rows live at row
        offset 18j; zeros elsewhere kill cross-tile terms).
      - batched activations: one sigmoid [128,6,48] + two tanh per group
        (the ~352cyc ACT pipeline fill made small per-tile activations the
        baseline bottleneck).
      - h-update writes straight into the next step's transpose input.
  * Pooling layer-1 decomposes: u1[i,j,:] = a[i,:] + b[j,:] (weight folding
    host-side); relu(a_i + b_j) one DVE/ACT op per robot pair; layer-2 on
    PE with block-diagonal [128,32] weight; neighbor mask folded into the
    PSUM accumulator via a -2^30 selection matmul; masked max-pool =
    tensor_reduce; pooled = relu(max + b_p2).
  * Fusion MLP on-device per core (24 robots).
"""

import numpy as np
import ml_dtypes
from contextlib import ExitStack

import concourse.bass as bass
import concourse.bacc as bacc
import concourse.tile as tile
from concourse import mybir
from concourse.bass_utils import run_bass_kernel_spmd

F32 = mybir.dt.float32
BF16 = mybir.dt.bfloat16
AL = mybir.AluOpType
AF = mybir.ActivationFunctionType
AX = mybir.AxisListType

T, N, B, A_DIM, H, EMB, MID, F = 8, 1536, 192, 2, 16, 16, 64, 256
NC_ = 8          # cores
BPC = B // NC_   # 24 robots per core
NPAIR = BPC // 2  # 12
NGRP = BPC // 8   # 3 robot groups of 8
BIG = float(2 ** 30)
CH = 512          # psum free chunk
NCH = N // CH     # 3
NT = N // 128     # 12 agent tiles
GT = 6            # tiles per LSTM group
TC = H + 2        # trin cols per tile (h16 + x2)
TW = GT * TC + 1  # 109: 6 tiles + ones col

bf16 = ml_dtypes.bfloat16
DUMMY_WARM = False

# blob column layout (bf16 [128, CB]); matmul operands all at partition 0
_C_BD = 0        # rows 0:128  [128, 32]
_C_WB4 = 32      # 3 x [96,128] bT2 weights, variant i nonzero at rows 32i:32i+19
_C_MSEL = 416    # rows 0:8    [8, 128]
_C_WCP = 544     # rows 0:2    [2, 64]
_C_PICE = 608    # rows 0:2    [2, 12]
_C_PICO = 620    # rows 0:2    [2, 12]
_C_WEMB = 632    # rows 0:4    [4, 16]
_C_SPT = 648    # rows 0:4    [4, 24]
_C_WFCA = 672    # rows 0:48   [48, 256]
CB = 928


def _din(nc, name, shape, dt):
    return nc.dram_tensor(name, list(shape), dt, kind="ExternalInput").ap()


WIW = NT * T * 2 + 128 + GT * 64 + NT * 2   # xa | identI | wall | pa columns

_IN_SPECS = [
    ("wiw", [128, WIW], BF16),
    ("nm8", [8, NGRP, N], BF16),
    ("blobB", [128, CB], BF16),
    ("blobF", [128, 2], F32),
    ("rpo", [16, BPC], BF16),
]


def _build():
    nc = bacc.Bacc("TRN2", target_bir_lowering=False, debug=False)
    a = {nm: _din(nc, nm, sh, dt) for nm, sh, dt in _IN_SPECS}
    a["out"] = nc.dram_tensor("out", [BPC, F], F32, kind="ExternalOutput").ap()
    with tile.TileContext(nc) as tc, ExitStack() as ctx:
        _emit(ctx, tc, nc, a)
    nc.compile()
    return nc


def _emit(ctx, tc, nc, a):
    sb = ctx.enter_context(tc.tile_pool(name="sb", bufs=1))

    # prefetch the sigmoid/tanh ACT table set immediately
    warm = sb.tile([1, 2], F32, name="warm")
    nc.vector.memset(warm, 0.0)
    nc.scalar.activation(out=warm, in_=warm, func=AF.Sigmoid)
    nc.scalar.activation(out=warm, in_=warm, func=AF.Tanh)

    # ---------- input DMAs ----------
    wiw = sb.tile([128, WIW], BF16, name="wiw")
    blob = sb.tile([128, CB], BF16, name="blob")
    nm_sb = sb.tile([8, NGRP, N], BF16, name="nm_sb")
    blobF = sb.tile([128, 2], F32, name="blobF")
    fuseT = sb.tile([48, BPC], BF16, name="fuseT")

    nc.sync.dma_start(out=wiw, in_=a["wiw"])
    nc.gpsimd.dma_start(out=blob, in_=a["blobB"])
    nc.gpsimd.dma_start(out=nm_sb, in_=a["nm8"])
    nc.sync.dma_start(out=blobF, in_=a["blobF"])
    nc.sync.dma_start(out=fuseT[16:32, :], in_=a["rpo"])

    xa = wiw[:, 0:NT * T * 2].rearrange("p (j t c) -> p j t c", t=T, c=2)
    identI = wiw[:, NT * T * 2:NT * T * 2 + 128]
    _cw = NT * T * 2 + 128
    wall = wiw[:, _cw:_cw + GT * 64].rearrange("p (j c) -> p j c", c=64)
    pa = wiw[:, _cw + GT * 64:WIW].rearrange("p (j c) -> p j c", c=2)

    BD_sb = blob[:, _C_BD:_C_BD + 32]
    msel = blob[0:8, _C_MSEL:_C_MSEL + 128]
    WcP_sb = blob[0:2, _C_WCP:_C_WCP + 64]
    pIcE_sb = blob[0:2, _C_PICE:_C_PICE + NPAIR]
    pIcO_sb = blob[0:2, _C_PICO:_C_PICO + NPAIR]
    W_emb_sb = blob[0:4, _C_WEMB:_C_WEMB + H]
    spT = blob[0:4, _C_SPT:_C_SPT + BPC]
    W_fca_sb = blob[0:48, _C_WFCA:_C_WFCA + F]
    b_embT = blobF[0:16, 0:1]
    b_p2T = blobF[32:48, 1:2]

    # ---------- LSTM state + staging (all bf16 SBUF) ----------
    trin = sb.tile([128, 2, 2, TW], BF16, name="trin")   # [agents, parity, grp, cols]
    c_sb = sb.tile([128, 2, GT, H], BF16, name="c_sb")
    # final-step staging: per tile 32-padded [hT(16); posT(2); ones(1)] so the
    # transposed form lands on 32-aligned row strips for the bT2 matmuls
    hfin = sb.tile([128, 2, GT, 32], BF16, name="hfin")
    nc.vector.memset(trin, 0.0)
    nc.vector.memset(trin[:, :, :, TW - 1:TW], 1.0)
    nc.vector.memset(c_sb, 0.0)
    nc.vector.memset(hfin, 0.0)
    nc.vector.memset(hfin[:, :, :, 18:19], 1.0)
    for g in range(2):
        nc.vector.tensor_copy(
            out=hfin[:, g, :, 16:18], in_=pa[:, GT * g:GT * g + GT, :])

    aT2 = sb.tile([128, NPAIR], F32, name="aT2")
    with tc.tile_pool(name="init_ps", bufs=1, space="PSUM") as ipool:
        a_ps = ipool.tile([128, NPAIR], F32, name="a_ps")
        nc.tensor.matmul(a_ps[0:64, :], WcP_sb, pIcE_sb, start=True, stop=True)
        nc.tensor.matmul(a_ps[64:128, :], WcP_sb, pIcO_sb, start=True, stop=True)
        nc.vector.tensor_copy(out=aT2, in_=a_ps)
        se_ps = ipool.tile([H, BPC], F32, name="se_ps")
        nc.tensor.matmul(se_ps, W_emb_sb, spT, start=True, stop=True)
        nc.scalar.activation(
            out=fuseT[0:16, :], in_=se_ps, func=AF.Relu, bias=b_embT)

    # ---------- LSTM over T steps, 2 pipelined groups of 6 tiles ----------
    # zearly: robot-group 0's first two mask-init chunks are matmul'd during
    # the LSTM (PE is idle-heavy there), so the pooling BD pipeline starts on
    # ru-arrival instead of waiting ~1.3us of cold mask matmuls.
    sgp = ctx.enter_context(tc.tile_pool(name="sgp", bufs=2))
    zearly = ctx.enter_context(
        tc.tile_pool(name="zearly", bufs=2, space="PSUM"))
    ze = [zearly.tile([128, CH], F32, name="ze", tag="z") for _ in range(2)]
    with tc.tile_pool(name="lstm_g", bufs=3, space="PSUM") as gpool, \
         tc.tile_pool(name="lstm_tp", bufs=2, space="PSUM") as tpool, \
         tc.tile_pool(name="lstm_tt", bufs=3) as ttpool:
        # prime step-0 x columns; later steps prefetch x mid-previous-step
        for g in range(2):
            hx = trin[:, 0, g, 0:GT * TC].rearrange("p (j c) -> p j c", c=TC)
            nc.vector.tensor_copy(
                out=hx[:, :, H:H + 2], in_=xa[:, GT * g:GT * g + GT, 0, :])
        for t in range(T):
            par, nxt = t % 2, (t + 1) % 2
            tps, tts, gps, sgs, tgs, ths, t1s, t2s = ([None, None] for _ in range(8))
            # PE: transposes then gates matmuls (both groups back-to-back)
            for g in range(2):
                tps[g] = tpool.tile([TW, 128], BF16, name="tp", tag="tp")
                nc.tensor.transpose(tps[g], trin[:, par, g, :], identI)
            for g in range(2):
                tts[g] = ttpool.tile([TW, 128], BF16, name="tt", tag="tt")
                nc.vector.tensor_copy(out=tts[g], in_=tps[g])
            # prefetch next step's x columns (off the next step's chain)
            if t + 1 < T:
                for g in range(2):
                    hx_n = trin[:, nxt, g, 0:GT * TC].rearrange(
                        "p (j c) -> p j c", c=TC)
                    nc.vector.tensor_copy(
                        out=hx_n[:, :, H:H + 2],
                        in_=xa[:, GT * g:GT * g + GT, t + 1, :])
            for g in range(2):
                gps[g] = gpool.tile([128, GT, 64], F32, name="g_ps", tag="g_ps")
                nc.tensor.matmul(
                    gps[g], tts[g], wall[0:TW, :, :], start=True, stop=True)
            if t in (2, 4):
                ch = t // 2 - 1
                nc.tensor.matmul(
                    ze[ch], msel, nm_sb[:, 0, CH * ch:CH * (ch + 1)],
                    start=True, stop=False, skip_group_check=True)
            # ACT: batched sigmoid first (sig_f unblocks DVE), then tanh(g)
            for g in range(2):
                sgs[g] = sgp.tile([128, GT, 48], BF16, name="sg", tag="sg")
                tgs[g] = sgp.tile([128, GT, H], BF16, name="tg", tag="tg")
                nc.scalar.activation(
                    out=sgs[g], in_=gps[g][:, :, 0:48], func=AF.Sigmoid)
                nc.scalar.activation(
                    out=tgs[g], in_=gps[g][:, :, 48:64], func=AF.Tanh)
            # DVE: c update (f*c first; i*g after tanh_g)
            for g in range(2):
                t1s[g] = sgp.tile([128, GT, H], BF16, name="t1", tag="t1")
                t2s[g] = sgp.tile([128, GT, H], BF16, name="t2", tag="t2")
                nc.vector.tensor_tensor(
                    out=t2s[g], in0=sgs[g][:, :, 16:32], in1=c_sb[:, g, :, :],
                    op=AL.mult)
                nc.vector.tensor_tensor(
                    out=t1s[g], in0=sgs[g][:, :, 0:16], in1=tgs[g], op=AL.mult)
                nc.vector.tensor_tensor(
                    out=c_sb[:, g, :, :], in0=t1s[g], in1=t2s[g], op=AL.add)
            # ACT: tanh(c); DVE: h -> next parity trin
            for g in range(2):
                ths[g] = sgp.tile([128, GT, H], BF16, name="th", tag="th")
                nc.scalar.activation(
                    out=ths[g], in_=c_sb[:, g, :, :], func=AF.Tanh)
            for g in range(2):
                if t == T - 1:
                    out_h = hfin[:, g, :, 0:H]
                else:
                    out_h = trin[:, nxt, g, 0:GT * TC].rearrange(
                        "p (j c) -> p j c", c=TC)[:, :, 0:H]
                nc.vector.tensor_tensor(
                    out=out_h, in0=sgs[g][:, :, 32:48], in1=ths[g],
                    op=AL.mult)

    # joint: 4 batched transposes of hfin (3 tiles each, 32-row strips),
    # then 12 row-strip matmuls compute bT2 directly (own PSUM scope so the
    # LSTM's gate pool banks are free here)
    bT2 = sb.tile([128, N], BF16, name="bT2")
    ttf = sb.tile([96, 4, 128], BF16, name="ttf")
    with tc.tile_pool(name="j_tp", bufs=2, space="PSUM") as jtpool, \
         tc.tile_pool(name="b_ps", bufs=2, space="PSUM") as bpool:
        for k in range(4):
            g, hf = k // 2, k % 2
            tp3 = jtpool.tile([96, 128], BF16, name="tp3", tag="tp")
            nc.tensor.transpose(
                tp3, hfin[:, g, 3 * hf:3 * hf + 3, :], identI)
            if k % 2 == 0:
                nc.vector.tensor_copy(out=ttf[:, k, :], in_=tp3)
            else:
                nc.scalar.copy(out=ttf[:, k, :], in_=tp3)
            b_ps = bpool.tile([128, 3 * 128], F32, name="b_ps")
            for i in range(3):
                nc.tensor.matmul(
                    b_ps[:, 128 * i:128 * (i + 1)],
                    blob[0:96, _C_WB4 + 128 * i:_C_WB4 + 128 * (i + 1)],
                    ttf[:, k, :],
                    start=True, stop=True, skip_group_check=True)
            if k % 2 == 0:
                nc.scalar.copy(
                    out=bT2[:, 384 * k:384 * (k + 1)], in_=b_ps)
            else:
                nc.vector.tensor_copy(
                    out=bT2[:, 384 * k:384 * (k + 1)], in_=b_ps)

    # ---------- pairwise pooling ----------
    pool_parts = sb.tile([128, NGRP], BF16, name="pool_parts")
    red3 = sb.tile([128, NGRP, NCH], F32, name="red3")
    def _emit_reduce(zs_g, g):
        for ch in range(NCH):
            nc.vector.tensor_reduce(
                out=red3[:, g, ch:ch + 1], in_=zs_g[ch], axis=AX.X,
                op=AL.max)
        nc.vector.tensor_reduce(
            out=pool_parts[:, g:g + 1], in_=red3[:, g, :], axis=AX.X,
            op=AL.max)

    with tc.tile_pool(name="ru_pool", bufs=8) as rupool, \
         tc.tile_pool(name="z_ps", bufs=6, space="PSUM") as zpool:
        zs_prev = None
        for g in range(NGRP):
            zs = []
            for ch in range(NCH):
                if g == 0 and ch < 2:
                    zs.append(ze[ch])   # mask-initialized during the LSTM
                    continue
                s = slice(CH * ch, CH * (ch + 1))
                zc = zpool.tile([128, CH], F32, name="z", tag="z")
                zs.append(zc)
                nc.tensor.matmul(
                    zc, msel, nm_sb[:, g, s], start=True, stop=False,
                    skip_group_check=True)
            for ai in range(4):
                p = 4 * g + ai
                ru = rupool.tile([128, N], BF16, name="ru", tag="ru")
                if ai < 2 or (g == 0 and ai == 2):
                    nc.vector.tensor_scalar(
                        out=ru, in0=bT2, scalar1=aT2[:, p:p + 1], scalar2=0.0,
                        op0=AL.add, op1=AL.max)
                else:
                    nc.scalar.activation(
                        out=ru, in_=bT2, func=AF.Relu,
                        bias=aT2[:, p:p + 1])
                for ch in range(NCH):
                    s = slice(CH * ch, CH * (ch + 1))
                    nc.tensor.matmul(
                        zs[ch][32 * ai:32 * (ai + 1), :], BD_sb, ru[:, s],
                        start=False, stop=(ai == 3),
                        tile_position=(0, 32 * ai), skip_group_check=True)
            # defer the previous group's reduces behind this group's rus so
            # they never head-of-line-block ready ru work on the DVE queue
            if zs_prev is not None:
                _emit_reduce(zs_prev, g - 1)
            zs_prev = zs
        _emit_reduce(zs_prev, NGRP - 1)
    # ---------- fusion MLP for this core's 24 robots ----------
    out_sb = sb.tile([BPC, F], F32, name="out_sb")
    with tc.tile_pool(name="f_ps", bufs=1, space="PSUM") as fpool:
        pg_ps = fpool.tile([48, BPC], F32, name="pg_ps")
        pg_v = pg_ps.rearrange("p (c l) -> p c l", l=8)
        for l in range(8):
            nc.tensor.matmul(
                pg_v[32:48, :, l], identI[:, 16 * l:16 * (l + 1)],
                pool_parts, start=True, stop=True)
        nc.scalar.activation(
            out=fuseT[32:48, :], in_=pg_ps[32:48, :], func=AF.Relu,
            bias=b_p2T)
        o_ps = fpool.tile([BPC, F], F32, name="o_ps")
        nc.tensor.matmul(o_ps, fuseT, W_fca_sb, start=True, stop=True)
        nc.vector.tensor_scalar(
            out=out_sb, in0=o_ps, scalar1=0.0, scalar2=None, op0=AL.max)
    nc.sync.dma_start(out=a["out"], in_=out_sb)


# ------------------------------------------------------------------
# host side
# ------------------------------------------------------------------
_NC_CACHE = None


def _gates_reorder(w):
    # torch gate order i,f,g,o (16 each) -> i,f,o,g
    i, f, g, o = np.split(np.asarray(w, np.float32), 4, axis=-1)
    return np.concatenate([i, f, o, g], axis=-1)


def _bf(x):
    return np.ascontiguousarray(np.asarray(x, np.float32).astype(bf16))


def _f32(x):
    return np.ascontiguousarray(np.asarray(x, np.float32))


def kernel(obs_traj_pos, traj_rel, neigh_index, robot_idx, r_goal, r_pose,
           action, W_he, b_he, W_ih, W_hh, b_ih, b_hh, W_sp, b_sp, W_p1, b_p1,
           W_p2, b_p2, W_emb, b_emb, W_fc, b_fc):
    global _NC_CACHE
    obs_traj_pos = np.asarray(obs_traj_pos, np.float32)
    traj_rel = np.asarray(traj_rel, np.float32)
    neigh_index = np.asarray(neigh_index)
    robot_idx = np.asarray(robot_idx)
    pos = obs_traj_pos[-1]                        # [N, 2]
    f = _f32

    # fold x-embedding into the recurrent matmul:
    #   gates = traj_rel@(W_he W_ih) + h@W_hh + (b_ih + b_he@W_ih + b_hh)
    W_heih = f(W_he) @ f(W_ih)
    bias = f(b_ih) + f(b_he) @ f(W_ih) + f(b_hh)
    W_cat = np.zeros((19, 64), np.float32)
    W_cat[0:16] = _gates_reorder(W_hh)
    W_cat[16:18] = _gates_reorder(W_heih)
    W_cat[18] = _gates_reorder(bias)

    Wc = f(W_sp) @ f(W_p1)[:EMB]                  # [2, 64]
    cvec = f(b_sp) @ f(W_p1)[:EMB] + f(b_p1)      # [64]
    # bT2 stationary: ench rows [h(16); posT(2); ones(1)]
    Wb2h = np.zeros((19, 64), np.float32)
    Wb2h[0:16] = f(W_p1)[EMB:]
    Wb2h[16:18] = -Wc
    Wb2h[18] = cvec
    Wb2 = np.concatenate([Wb2h, Wb2h], axis=1)    # [19, 128]

    # block-column gate weights: tile j's rows at 18j (zeros elsewhere
    # kill cross-tile terms of the shared transposed lhsT)
    wall0 = np.zeros((128, GT, 64), np.float32)
    for j in range(GT):
        wall0[TC * j:TC * j + 18, j, :] = W_cat[0:18]
        wall0[TW - 1, j, :] = W_cat[18]

    blob0 = np.zeros((128, CB), np.float32)
    bd = np.zeros((128, 32), np.float32)
    bd[0:64, 0:16] = W_p2
    bd[64:128, 16:32] = W_p2
    blob0[:, _C_BD:_C_BD + 32] = bd
    for i in range(3):
        blob0[32 * i:32 * i + 19, _C_WB4 + 128 * i:_C_WB4 + 128 * (i + 1)] = Wb2
    ms = np.zeros((8, 128), np.float32)
    for l in range(8):
        ms[l, 16 * l:16 * (l + 1)] = -BIG
    blob0[0:8, _C_MSEL:_C_MSEL + 128] = ms
    blob0[0:2, _C_WCP:_C_WCP + 64] = Wc
    blob0[0:4, _C_WEMB:_C_WEMB + H] = W_emb
    wf = np.zeros((48, F), np.float32)
    wf[0:16] = W_fc[0:16]        # spatial_emb rows
    wf[16:21] = W_fc[32:37]      # r_pose rows
    wf[21] = b_fc                # bias row (matched by ones in rpo row 5->21)
    wf[32:48] = W_fc[16:32]      # pooled rows
    blob0[0:48, _C_WFCA:_C_WFCA + F] = wf

    # agent-major traj_rel: xa[a, j, t, :] = traj_rel[t, 128j+a, :]
    xa = np.transpose(traj_rel.reshape(T, NT, 128, 2), (2, 1, 0, 3))
    pa = np.transpose(pos.reshape(NT, 128, 2), (1, 0, 2))  # [128, NT, 2]
    blobF0 = np.zeros((128, 2), np.float32)
    blobF0[0:16, 0] = f(b_emb)
    blobF0[32:48, 1] = f(b_p2)

    in_maps = []
    for c in range(NC_):
        I = robot_idx[BPC * c:BPC * (c + 1)]
        nm = np.zeros((8, NGRP, N), np.float32)
        for g in range(NGRP):
            for l in range(8):
                nm[l, g] = 1.0 - (neigh_index[I[8 * g + l]] > 0)
        blobc = blob0.copy()
        blobc[0:2, _C_PICE:_C_PICE + NPAIR] = pos[I[0::2]].T
        blobc[0:2, _C_PICO:_C_PICO + NPAIR] = pos[I[1::2]].T
        spt = np.zeros((4, BPC), np.float32)
        spt[0:2] = (f(r_goal)[BPC * c:BPC * (c + 1)] - pos[I]).T
        spt[2:4] = f(action)[BPC * c:BPC * (c + 1)].T
        blobc[0:4, _C_SPT:_C_SPT + BPC] = spt
        rpo = np.zeros((16, BPC), np.float32)
        rpo[0:5] = f(r_pose)[BPC * c:BPC * (c + 1)].T
        rpo[5] = 1.0
        wiw = np.concatenate(
            [xa.reshape(128, NT * T * 2), np.eye(128, dtype=np.float32),
             wall0.reshape(128, GT * 64), pa.reshape(128, NT * 2)], axis=1)
        in_maps.append(dict(
            wiw=_bf(wiw),
            nm8=_bf(nm),
            blobB=_bf(blobc),
            blobF=blobF0,
            rpo=_bf(rpo),
        ))

    if _NC_CACHE is None:
        _NC_CACHE = _build()
    res = run_bass_kernel_spmd(_NC_CACHE, in_maps, core_ids=list(range(NC_)))
    out = np.concatenate([r["out"] for r in res.results], axis=0)
    return out.astype(np.float32)


if __name__ == "__main__":
    import reference
    inp = {k: np.asarray(v) for k, v in reference.setup_inputs().items()}
    got = kernel(**inp)
    exp = np.asarray(reference.reference(**inp))
    err = np.abs(got - exp)
    print("max abs err", err.max(), "scale", np.abs(exp).max())
    print("rel-of-max", err.max() / np.abs(exp).max())


# revision 107
# speedup vs baseline: 1.0213x; 1.0213x over previous
"""Trainium2 Bass kernel for CustomEncoderWithAction (gnn_message_passing).

Strategy (8 NeuronCores, full inputs in / full output out):
  * Only pooled[robot_idx] (B=192 rows) is consumed downstream, so the
    [N,N] pairwise pooling is computed ONLY for the 192 robot agents,
    sharded 24 per core.
  * LSTM encoder (T=8, all N agents) replicated on every core, software-
    pipelined as 2 groups x 6 agent-tiles:
      - per group+step ONE gates matmul: stationary lhsT = transposed
        [h|x|1] block (one PE transpose per group), moving rhs = [109,384]
        block-column weight matrix (tile j's W_hh/W_xh rows live at row
        offset 18j; zeros elsewhere kill cross-tile terms).
      - batched activations: one sigmoid [128,6,48] + two tanh per group
        (the ~352cyc ACT pipeline fill made small per-tile activations the
        baseline bottleneck).
      - h-update writes straight into the next step's transpose input.
  * Pooling layer-1 decomposes: u1[i,j,:] = a[i,:] + b[j,:] (weight folding
    host-side); relu(a_i + b_j) one DVE/ACT op per robot pair; layer-2 on
    PE with block-diagonal [128,32] weight; neighbor mask folded into the
    PSUM accumulator via a -2^30 selection matmul; masked max-pool =
    tensor_reduce; pooled = relu(max + b_p2).
  * Fusion MLP on-device per core (24 robots).
"""

import numpy as np
import ml_dtypes
from contextlib import ExitStack

import concourse.bass as bass
import concourse.bacc as bacc
import concourse.tile as tile
from concourse import mybir
from concourse.bass_utils import run_bass_kernel_spmd

F32 = mybir.dt.float32
BF16 = mybir.dt.bfloat16
AL = mybir.AluOpType
AF = mybir.ActivationFunctionType
AX = mybir.AxisListType

T, N, B, A_DIM, H, EMB, MID, F = 8, 1536, 192, 2, 16, 16, 64, 256
NC_ = 8          # cores
BPC = B // NC_   # 24 robots per core
NPAIR = BPC // 2  # 12
NGRP = BPC // 8   # 3 robot groups of 8
BIG = float(2 ** 30)
CH = 512          # psum free chunk
NCH = N // CH     # 3
NT = N // 128     # 12 agent tiles
GT = 6            # tiles per LSTM group
TC = H + 2        # trin cols per tile (h16 + x2)
TW = GT * TC + 1  # 109: 6 tiles + ones col

bf16 = ml_dtypes.bfloat16
DUMMY_WARM = False

# blob column layout (bf16 [128, CB]); matmul operands all at partition 0
_C_BD = 0        # rows 0:128  [128, 32]
_C_WB4 = 32      # 3 x [96,128] bT2 weights, variant i nonzero at rows 32i:32i+19
_C_MSEL = 416    # rows 0:8    [8, 128]
_C_WCP = 544     # rows 0:2    [2, 64]
_C_PICE = 608    # rows 0:2    [2, 12]
_C_PICO = 620    # rows 0:2    [2, 12]
_C_WEMB = 632    # rows 0:4    [4, 16]
_C_SPT = 648    # rows 0:4    [4, 24]
_C_WFCA = 672    # rows 0:48   [48, 256]
CB = 928


def _din(nc, name, shape, dt):
    return nc.dram_tensor(name, list(shape), dt, kind="ExternalInput").ap()


WIW = NT * T * 2 + 128 + GT * 64 + NT * 2   # xa | identI | wall | pa columns

_IN_SPECS = [
    ("wiw", [128, WIW], BF16),
    ("nm8", [8, NGRP, N], BF16),
    ("blobB", [128, CB], BF16),
    ("blobF", [128, 2], F32),
    ("rpo", [16, BPC], BF16),
]


def _build():
    nc = bacc.Bacc("TRN2", target_bir_lowering=False, debug=False)
    a = {nm: _din(nc, nm, sh, dt) for nm, sh, dt in _IN_SPECS}
    a["out"] = nc.dram_tensor("out", [BPC, F], F32, kind="ExternalOutput").ap()
    with tile.TileContext(nc) as tc, ExitStack() as ctx:
        _emit(ctx, tc, nc, a)
    nc.compile()
    return nc


def _emit(ctx, tc, nc, a):
    sb = ctx.enter_context(tc.tile_pool(name="sb", bufs=1))

    # prefetch the sigmoid/tanh ACT table set immediately
    warm = sb.tile([1, 2], F32, name="warm")
    nc.vector.memset(warm, 0.0)
    nc.scalar.activation(out=warm, in_=warm, func=AF.Sigmoid)
    nc.scalar.activation(out=warm, in_=warm, func=AF.Tanh)

    # ---------- input DMAs ----------
    wiw = sb.tile([128, WIW], BF16, name="wiw")
    blob = sb.tile([128, CB], BF16, name="blob")
    nm_sb = sb.tile([8, NGRP, N], BF16, name="nm_sb")
    blobF = sb.tile([128, 2], F32, name="blobF")
    fuseT = sb.tile([48, BPC], BF16, name="fuseT")

    nc.sync.dma_start(out=wiw, in_=a["wiw"])
    nc.gpsimd.dma_start(out=blob, in_=a["blobB"])
    nc.gpsimd.dma_start(out=nm_sb, in_=a["nm8"])
    nc.sync.dma_start(out=blobF, in_=a["blobF"])
    nc.sync.dma_start(out=fuseT[16:32, :], in_=a["rpo"])

    xa = wiw[:, 0:NT * T * 2].rearrange("p (j t c) -> p j t c", t=T, c=2)
    identI = wiw[:, NT * T * 2:NT * T * 2 + 128]
    _cw = NT * T * 2 + 128
    wall = wiw[:, _cw:_cw + GT * 64].rearrange("p (j c) -> p j c", c=64)
    pa = wiw[:, _cw + GT * 64:WIW].rearrange("p (j c) -> p j c", c=2)

    BD_sb = blob[:, _C_BD:_C_BD + 32]
    msel = blob[0:8, _C_MSEL:_C_MSEL + 128]
    WcP_sb = blob[0:2, _C_WCP:_C_WCP + 64]
    pIcE_sb = blob[0:2, _C_PICE:_C_PICE + NPAIR]
    pIcO_sb = blob[0:2, _C_PICO:_C_PICO + NPAIR]
    W_emb_sb = blob[0:4, _C_WEMB:_C_WEMB + H]
    spT = blob[0:4, _C_SPT:_C_SPT + BPC]
    W_fca_sb = blob[0:48, _C_WFCA:_C_WFCA + F]
    b_embT = blobF[0:16, 0:1]
    b_p2T = blobF[32:48, 1:2]

    # ---------- LSTM state + staging (all bf16 SBUF) ----------
    trin = sb.tile([128, 2, 2, TW], BF16, name="trin")   # [agents, parity, grp, cols]
    c_sb = sb.tile([128, 2, GT, H], BF16, name="c_sb")
    # final-step staging: per tile 32-padded [hT(16); posT(2); ones(1)] so the
    # transposed form lands on 32-aligned row strips for the bT2 matmuls
    hfin = sb.tile([128, 2, GT, 32], BF16, name="hfin")
    nc.vector.memset(trin, 0.0)
    nc.vector.memset(trin[:, :, :, TW - 1:TW], 1.0)
    nc.vector.memset(c_sb, 0.0)
    nc.vector.memset(hfin, 0.0)
    nc.vector.memset(hfin[:, :, :, 18:19], 1.0)
    for g in range(2):
        nc.vector.tensor_copy(
            out=hfin[:, g, :, 16:18], in_=pa[:, GT * g:GT * g + GT, :])

    aT2 = sb.tile([128, NPAIR], F32, name="aT2")
    with tc.tile_pool(name="init_ps", bufs=1, space="PSUM") as ipool:
        a_ps = ipool.tile([128, NPAIR], F32, name="a_ps")
        nc.tensor.matmul(a_ps[0:64, :], WcP_sb, pIcE_sb, start=True, stop=True)
        nc.tensor.matmul(a_ps[64:128, :], WcP_sb, pIcO_sb, start=True, stop=True)
        nc.vector.tensor_copy(out=aT2, in_=a_ps)
        se_ps = ipool.tile([H, BPC], F32, name="se_ps")
        nc.tensor.matmul(se_ps, W_emb_sb, spT, start=True, stop=True)
        nc.scalar.activation(
            out=fuseT[0:16, :], in_=se_ps, func=AF.Relu, bias=b_embT)

    # ---------- LSTM over T steps, 2 pipelined groups of 6 tiles ----------
    # zearly: robot-group 0's first two mask-init chunks are matmul'd during
    # the LSTM (PE is idle-heavy there), so the pooling BD pipeline starts on
    # ru-arrival instead of waiting ~1.3us of cold mask matmuls.
    sgp = ctx.enter_context(tc.tile_pool(name="sgp", bufs=2))
    zearly = ctx.enter_context(
        tc.tile_pool(name="zearly", bufs=2, space="PSUM"))
    ze = [zearly.tile([128, CH], F32, name="ze", tag="z") for _ in range(2)]
    with tc.tile_pool(name="lstm_g", bufs=3, space="PSUM") as gpool, \
         tc.tile_pool(name="lstm_tp", bufs=2, space="PSUM") as tpool, \
         tc.tile_pool(name="lstm_tt", bufs=3) as ttpool:
        # prime step-0 x columns; later steps prefetch x mid-previous-step
        for g in range(2):
            hx = trin[:, 0, g, 0:GT * TC].rearrange("p (j c) -> p j c", c=TC)
            nc.vector.tensor_copy(
                out=hx[:, :, H:H + 2], in_=xa[:, GT * g:GT * g + GT, 0, :])
        for t in range(T):
            par, nxt = t % 2, (t + 1) % 2
            tps, tts, gps, sgs, tgs, ths, t1s, t2s = ([None, None] for _ in range(8))
            # PE: transposes then gates matmuls (both groups back-to-back)
            for g in range(2):
                tps[g] = tpool.tile([TW, 128], BF16, name="tp", tag="tp")
                nc.tensor.transpose(tps[g], trin[:, par, g, :], identI)
            for g in range(2):
                tts[g] = ttpool.tile([TW, 128], BF16, name="tt", tag="tt")
                nc.vector.tensor_copy(out=tts[g], in_=tps[g])
            # prefetch next step's x columns (off the next step's chain)
            if t + 1 < T:
                for g in range(2):
                    hx_n = trin[:, nxt, g, 0:GT * TC].rearrange(
                        "p (j c) -> p j c", c=TC)
                    nc.vector.tensor_copy(
                        out=hx_n[:, :, H:H + 2],
                        in_=xa[:, GT * g:GT * g + GT, t + 1, :])
            for g in range(2):
                gps[g] = gpool.tile([128, GT, 64], F32, name="g_ps", tag="g_ps")
                nc.tensor.matmul(
                    gps[g], tts[g], wall[0:TW, :, :], start=True, stop=True)
            if t in (2, 4):
                ch = t // 2 - 1
                nc.tensor.matmul(
                    ze[ch], msel, nm_sb[:, 0, CH * ch:CH * (ch + 1)],
                    start=True, stop=False, skip_group_check=True)
            # ACT: batched sigmoid first (sig_f unblocks DVE), then tanh(g)
            for g in range(2):
                sgs[g] = sgp.tile([128, GT, 48], BF16, name="sg", tag="sg")
                tgs[g] = sgp.tile([128, GT, H], BF16, name="tg", tag="tg")
                nc.scalar.activation(
                    out=sgs[g], in_=gps[g][:, :, 0:48], func=AF.Sigmoid)
                nc.scalar.activation(
                    out=tgs[g], in_=gps[g][:, :, 48:64], func=AF.Tanh)
            # DVE: c update (f*c first; i*g after tanh_g)
            for g in range(2):
                t1s[g] = sgp.tile([128, GT, H], BF16, name="t1", tag="t1")
                t2s[g] = sgp.tile([128, GT, H], BF16, name="t2", tag="t2")
                nc.vector.tensor_tensor(
                    out=t2s[g], in0=sgs[g][:, :, 16:32], in1=c_sb[:, g, :, :],
                    op=AL.mult)
                nc.vector.tensor_tensor(
                    out=t1s[g], in0=sgs[g][:, :, 0:16], in1=tgs[g], op=AL.mult)
                nc.vector.tensor_tensor(
                    out=c_sb[:, g, :, :], in0=t1s[g], in1=t2s[g], op=AL.add)
            # ACT: tanh(c); DVE: h -> next parity trin
            for g in range(2):
                ths[g] = sgp.tile([128, GT, H], BF16, name="th", tag="th")
                nc.scalar.activation(
                    out=ths[g], in_=c_sb[:, g, :, :], func=AF.Tanh)
            for g in range(2):
                if t == T - 1:
                    out_h = hfin[:, g, :, 0:H]
                else:
                    out_h = trin[:, nxt, g, 0:GT * TC].rearrange(
                        "p (j c) -> p j c", c=TC)[:, :, 0:H]
                nc.vector.tensor_tensor(
                    out=out_h, in0=sgs[g][:, :, 32:48], in1=ths[g],
                    op=AL.mult)

    # joint: 4 batched transposes of hfin (3 tiles each, 32-row strips),
    # then 12 row-strip matmuls compute bT2 directly (own PSUM scope so the
    # LSTM's gate pool banks are free here)
    bT2 = sb.tile([128, N], BF16, name="bT2")
    ttf = sb.tile([96, 4, 128], BF16, name="ttf")
    with tc.tile_pool(name="j_tp", bufs=2, space="PSUM") as jtpool, \
         tc.tile_pool(name="b_ps", bufs=2, space="PSUM") as bpool:
        for k in range(4):
            g, hf = k // 2, k % 2
            tp3 = jtpool.tile([96, 128], BF16, name="tp3", tag="tp")
            nc.tensor.transpose(
                tp3, hfin[:, g, 3 * hf:3 * hf + 3, :], identI)
            if k % 2 == 0:
                nc.vector.tensor_copy(out=ttf[:, k, :], in_=tp3)
            else:
                nc.scalar.copy(out=ttf[:, k, :], in_=tp3)
            b_ps = bpool.tile([128, 3 * 128], F32, name="b_ps")
            for i in range(3):
                nc.tensor.matmul(
                    b_ps[:, 128 * i:128 * (i + 1)],
                    blob[0:96, _C_WB4 + 128 * i:_C_WB4 + 128 * (i + 1)],
                    ttf[:, k, :],
                    start=True, stop=True, skip_group_check=True)
            if k % 2 == 0:
                nc.scalar.copy(
                    out=bT2[:, 384 * k:384 * (k + 1)], in_=b_ps)
            else:
                nc.vector.tensor_copy(
                    out=bT2[:, 384 * k:384 * (k + 1)], in_=b_ps)

    # ---------- pairwise pooling ----------
    pool_parts = sb.tile([128, NGRP], BF16, name="pool_parts")
    red3 = sb.tile([128, NGRP, NCH], F32, name="red3")
    def _emit_reduce(zs_g, g):
        for ch in range(NCH):
            nc.vector.tensor_reduce(
                out=red3[:, g, ch:ch + 1], in_=zs_g[ch], axis=AX.X,
                op=AL.max)
        nc.vector.tensor_reduce(
            out=pool_parts[:, g:g + 1], in_=red3[:, g, :], axis=AX.X,
            op=AL.max)

    with tc.tile_pool(name="ru_pool", bufs=6) as rupool, \
         tc.tile_pool(name="z_ps", bufs=6, space="PSUM") as zpool:
        zs_prev = None
        for g in range(NGRP):
            zs = []
            for ch in range(NCH):
                if g == 0 and ch < 2:
                    zs.append(ze[ch])   # mask-initialized during the LSTM
                    continue
                s = slice(CH * ch, CH * (ch + 1))
                zc = zpool.tile([128, CH], F32, name="z", tag="z")
                zs.append(zc)
                nc.tensor.matmul(
                    zc, msel, nm_sb[:, g, s], start=True, stop=False,
                    skip_group_check=True)
            for ai in range(4):
                p = 4 * g + ai
                ru = rupool.tile([128, N], BF16, name="ru", tag="ru")
                if ai < 2 or (g == 0 and ai == 2):
                    nc.vector.tensor_scalar(
                        out=ru, in0=bT2, scalar1=aT2[:, p:p + 1], scalar2=0.0,
                        op0=AL.add, op1=AL.max)
                else:
                    nc.scalar.activation(
                        out=ru, in_=bT2, func=AF.Relu,
                        bias=aT2[:, p:p + 1])
                for ch in range(NCH):
                    s = slice(CH * ch, CH * (ch + 1))
                    nc.tensor.matmul(
                        zs[ch][32 * ai:32 * (ai + 1), :], BD_sb, ru[:, s],
                        start=False, stop=(ai == 3),
                        tile_position=(0, 32 * ai), skip_group_check=True)
            # defer the previous group's reduces behind this group's rus so
            # they never head-of-line-block ready ru work on the DVE queue
            if zs_prev is not None:
                _emit_reduce(zs_prev, g - 1)
            zs_prev = zs
        _emit_reduce(zs_prev, NGRP - 1)
    # ---------- fusion MLP for this core's 24 robots ----------
    out_sb = sb.tile([BPC, F], F32, name="out_sb")
    with tc.tile_pool(name="f_ps", bufs=1, space="PSUM") as fpool:
        pg_ps = fpool.tile([48, BPC], F32, name="pg_ps")
        pg_v = pg_ps.rearrange("p (c l) -> p c l", l=8)
        for l in range(8):
            nc.tensor.matmul(
                pg_v[32:48, :, l], identI[:, 16 * l:16 * (l + 1)],
                pool_parts, start=True, stop=True)
        nc.scalar.activation(
            out=fuseT[32:48, :], in_=pg_ps[32:48, :], func=AF.Relu,
            bias=b_p2T)
        o_ps = fpool.tile([BPC, F], F32, name="o_ps")
        nc.tensor.matmul(o_ps, fuseT, W_fca_sb, start=True, stop=True)
        nc.vector.tensor_scalar(
            out=out_sb, in0=o_ps, scalar1=0.0, scalar2=None, op0=AL.max)
    nc.sync.dma_start(out=a["out"], in_=out_sb)


# ------------------------------------------------------------------
# host side
# ------------------------------------------------------------------
_NC_CACHE = None


def _gates_reorder(w):
    # torch gate order i,f,g,o (16 each) -> i,f,o,g
    i, f, g, o = np.split(np.asarray(w, np.float32), 4, axis=-1)
    return np.concatenate([i, f, o, g], axis=-1)


def _bf(x):
    return np.ascontiguousarray(np.asarray(x, np.float32).astype(bf16))


def _f32(x):
    return np.ascontiguousarray(np.asarray(x, np.float32))


def kernel(obs_traj_pos, traj_rel, neigh_index, robot_idx, r_goal, r_pose,
           action, W_he, b_he, W_ih, W_hh, b_ih, b_hh, W_sp, b_sp, W_p1, b_p1,
           W_p2, b_p2, W_emb, b_emb, W_fc, b_fc):
    global _NC_CACHE
    obs_traj_pos = np.asarray(obs_traj_pos, np.float32)
    traj_rel = np.asarray(traj_rel, np.float32)
    neigh_index = np.asarray(neigh_index)
    robot_idx = np.asarray(robot_idx)
    pos = obs_traj_pos[-1]                        # [N, 2]
    f = _f32

    # fold x-embedding into the recurrent matmul:
    #   gates = traj_rel@(W_he W_ih) + h@W_hh + (b_ih + b_he@W_ih + b_hh)
    W_heih = f(W_he) @ f(W_ih)
    bias = f(b_ih) + f(b_he) @ f(W_ih) + f(b_hh)
    W_cat = np.zeros((19, 64), np.float32)
    W_cat[0:16] = _gates_reorder(W_hh)
    W_cat[16:18] = _gates_reorder(W_heih)
    W_cat[18] = _gates_reorder(bias)

    Wc = f(W_sp) @ f(W_p1)[:EMB]                  # [2, 64]
    cvec = f(b_sp) @ f(W_p1)[:EMB] + f(b_p1)      # [64]
    # bT2 stationary: ench rows [h(16); posT(2); ones(1)]
    Wb2h = np.zeros((19, 64), np.float32)
    Wb2h[0:16] = f(W_p1)[EMB:]
    Wb2h[16:18] = -Wc
    Wb2h[18] = cvec
    Wb2 = np.concatenate([Wb2h, Wb2h], axis=1)    # [19, 128]

    # block-column gate weights: tile j's rows at 18j (zeros elsewhere
    # kill cross-tile terms of the shared transposed lhsT)
    wall0 = np.zeros((128, GT, 64), np.float32)
    for j in range(GT):
        wall0[TC * j:TC * j + 18, j, :] = W_cat[0:18]
        wall0[TW - 1, j, :] = W_cat[18]

    blob0 = np.zeros((128, CB), np.float32)
    bd = np.zeros((128, 32), np.float32)
    bd[0:64, 0:16] = W_p2
    bd[64:128, 16:32] = W_p2
    blob0[:, _C_BD:_C_BD + 32] = bd
    for i in range(3):
        blob0[32 * i:32 * i + 19, _C_WB4 + 128 * i:_C_WB4 + 128 * (i + 1)] = Wb2
    ms = np.zeros((8, 128), np.float32)
    for l in range(8):
        ms[l, 16 * l:16 * (l + 1)] = -BIG
    blob0[0:8, _C_MSEL:_C_MSEL + 128] = ms
    blob0[0:2, _C_WCP:_C_WCP + 64] = Wc
    blob0[0:4, _C_WEMB:_C_WEMB + H] = W_emb
    wf = np.zeros((48, F), np.float32)
    wf[0:16] = W_fc[0:16]        # spatial_emb rows
    wf[16:21] = W_fc[32:37]      # r_pose rows
    wf[21] = b_fc                # bias row (matched by ones in rpo row 5->21)
    wf[32:48] = W_fc[16:32]      # pooled rows
    blob0[0:48, _C_WFCA:_C_WFCA + F] = wf

    # agent-major traj_rel: xa[a, j, t, :] = traj_rel[t, 128j+a, :]
    xa = np.transpose(traj_rel.reshape(T, NT, 128, 2), (2, 1, 0, 3))
    pa = np.transpose(pos.reshape(NT, 128, 2), (1, 0, 2))  # [128, NT, 2]
    blobF0 = np.zeros((128, 2), np.float32)
    blobF0[0:16, 0] = f(b_emb)
    blobF0[32:48, 1] = f(b_p2)

    in_maps = []
    for c in range(NC_):
        I = robot_idx[BPC * c:BPC * (c + 1)]
        nm = np.zeros((8, NGRP, N), np.float32)
        for g in range(NGRP):
            for l in range(8):
                nm[l, g] = 1.0 - (neigh_index[I[8 * g + l]] > 0)
        blobc = blob0.copy()
        blobc[0:2, _C_PICE:_C_PICE + NPAIR] = pos[I[0::2]].T
        blobc[0:2, _C_PICO:_C_PICO + NPAIR] = pos[I[1::2]].T
        spt = np.zeros((4, BPC), np.float32)
        spt[0:2] = (f(r_goal)[BPC * c:BPC * (c + 1)] - pos[I]).T
        spt[2:4] = f(action)[BPC * c:BPC * (c + 1)].T
        blobc[0:4, _C_SPT:_C_SPT + BPC] = spt
        rpo = np.zeros((16, BPC), np.float32)
        rpo[0:5] = f(r_pose)[BPC * c:BPC * (c + 1)].T
        rpo[5] = 1.0
        wiw = np.concatenate(
            [xa.reshape(128, NT * T * 2), np.eye(128, dtype=np.float32),
             wall0.reshape(128, GT * 64), pa.reshape(128, NT * 2)], axis=1)
        in_maps.append(dict(
            wiw=_bf(wiw),
            nm8=_bf(nm),
            blobB=_bf(blobc),
            blobF=blobF0,
            rpo=_bf(rpo),
        ))

    if _NC_CACHE is None:
        _NC_CACHE = _build()
    res = run_bass_kernel_spmd(_NC_CACHE, in_maps, core_ids=list(range(NC_)))
    out = np.concatenate([r["out"] for r in res.results], axis=0)
    return out.astype(np.float32)


if __name__ == "__main__":
    import reference
    inp = {k: np.asarray(v) for k, v in reference.setup_inputs().items()}
    got = kernel(**inp)
    exp = np.asarray(reference.reference(**inp))
    err = np.abs(got - exp)
    print("max abs err", err.max(), "scale", np.abs(exp).max())
    print("rel-of-max", err.max() / np.abs(exp).max())
